# revision 14
# baseline (speedup 1.0000x reference)
"""Trainium2 Bass kernel for nn_ConvProjector (conv3x3 -> ReLU -> conv3x3 -> ReLU
-> adaptive-avg-pool upsample 32x32 -> 687x1024 -> 1x1 conv 256->24 + bias).

Strategy (v3):
  * Pool and 1x1 conv commute: reduce 256->24 channels at 32x32 first, then
    upsample only 24 channels.
  * W axis: 1024 = 32*32 -> pure replication via a 0/1 expansion matmul.
  * H axis: 687 from 32 -> 21/22-row runs; replicated rows DMA'd from a
    4x-materialized SBUF buffer (8KB descriptors), averaged boundary rows
    via a second accumulating expansion matmul scaled 0.5.
  * Sharding: core k owns input rows 4k..4k+3 (+halos); no collectives.
  * The HWDGE descriptor generators (~60M desc/s per queue) are the real
    bottleneck, so every large DMA uses multi-KB descriptors: w1 as two
    multi-tap blocks (10/8KB), w2 as two blocks (6/3KB), output writes from
    a 4x-replicated buffer (8KB) plus a 2-row tail buffer (4KB).  All the
    tiny per-partition tensors (biases via f32<->2xf16 bitcast, 1x1 weights,
    row mask) ride in one combo tile.
  * PE warmed with dummy matmuls during the initial DMA wait; back-end
    chunked in two (rows 0,1 | rows 2,3) so chunk 1 compute hides under
    chunk 0's write; ReLU/bias and the 0.5*(a+b)+bias all run on the DVE
    where that shortens the critical tail.
Output is assembled on the host from the per-core (4, 24, 22, 1024) buffers.
"""
import sys

if '/opt/trn_rl_repo' not in sys.path:
    sys.path.insert(0, '/opt/trn_rl_repo')

import numpy as np

IN_C, MID_C, OUT_C = 576, 256, 24
H = W = 32
OUT_H, OUT_W = 687, 1024
NCORES = 8
P = 128
KC1 = 4           # full 128-channel input chunks for conv1 (plus one 64 chunk)
KC2 = 2           # 256/128 chunks for conv2 / 1x1
MC = 2            # 256/128 output-channel chunks for conv1/conv2
W36 = 36          # padded row width (2 zero cols each side)
RX, R1, R2 = 9, 7, 5          # x rows / h1 rows / h2 (=r) rows per core
XBLK = RX * W36               # 324  per-kc x block
XSLACK = 16                   # rhs overrun slack so N can pad to 256
N1 = 256                      # conv1 matmul N (covers the 7 h1 rows)
H1BLK = R1 * W36              # 252  per-mc h1 block
H1SLACK = 80
N2 = 180                      # conv2 matmul N (covers the 5 h2 rows)
NV2 = 176                     # valid h2 flat span per mc
RUN = 22                      # output rows per owned input row in core buffer
CB = 66                       # combo tile cols: 10 bias(f16 pairs) + 48 wr + 7 mask + pad

_prog_cache = {}


def _h_runs():
    i = np.arange(OUT_H)
    s = (i * H) // OUT_H
    t = np.searchsorted(s, np.arange(H + 1), side='left')
    return s, t


def _build_program():
    import concourse.bass as bass
    import concourse.bacc as bacc
    import concourse.mybir as mybir
    from concourse.tile import TileContext

    f32 = mybir.dt.float32
    f16 = mybir.dt.float16
    Alu = mybir.AluOpType
    nc = bacc.Bacc("TRN2", target_bir_lowering=False, debug=False,
                   num_devices=NCORES)

    TB1 = MC * P                  # 256: per-(tap,kc) lhsT block
    W1A, W1B = 5, 4               # conv1 taps 0-4 / 5-8 block split
    TB2 = KC2 * MC * P            # 512: per-tap conv2 block
    W2A, W2B = 6, 3               # conv2 taps 0-5 / 6-8 block split

    xs_d = nc.dram_tensor("xs", [P, KC1 * XBLK + XSLACK], f16, kind="ExternalInput")
    xh_d = nc.dram_tensor("xh", [64, XBLK + XSLACK], f16, kind="ExternalInput")
    w1a_d = nc.dram_tensor("w1a", [P, W1A * KC1 * TB1], f16, kind="ExternalInput")
    w1b_d = nc.dram_tensor("w1b", [P, W1B * KC1 * TB1], f16, kind="ExternalInput")
    wh_d = nc.dram_tensor("whp", [64, 9 * TB1], f16, kind="ExternalInput")
    w2a_d = nc.dram_tensor("w2a", [P, W2A * TB2], f16, kind="ExternalInput")
    w2b_d = nc.dram_tensor("w2b", [P, W2B * TB2], f16, kind="ExternalInput")
    cb_d = nc.dram_tensor("cbp", [P, CB], f16, kind="ExternalInput")
    em_d = nc.dram_tensor("emp", [32, OUT_W], f16, kind="ExternalInput")
    out_d = nc.dram_tensor("outb", [4 * OUT_C * RUN, OUT_W], f16,
                           kind="ExternalOutput")

    Relu = mybir.ActivationFunctionType.Relu
    Ident = mybir.ActivationFunctionType.Identity

    with TileContext(nc) as tc:
        with (
            tc.tile_pool(name="sb", bufs=1) as sb,
            tc.tile_pool(name="ps", bufs=1, space="PSUM") as psp,
        ):
            wz_t = sb.tile([P, N1], f16)
            x_t = sb.tile([P, KC1 * XBLK + XSLACK], f16)
            xh_t = sb.tile([64, XBLK + XSLACK], f16)
            w1a_t = sb.tile([P, W1A * KC1 * TB1], f16)
            w1b_t = sb.tile([P, W1B * KC1 * TB1], f16)
            wh_t = sb.tile([64, 9 * TB1], f16)
            w2a_t = sb.tile([P, W2A * TB2], f16)
            w2b_t = sb.tile([P, W2B * TB2], f16)
            cb_t = sb.tile([P, CB], f16)
            em_t = sb.tile([32, OUT_W], f16)
            h1_t = sb.tile([P, MC * H1BLK + H1SLACK], f16)
            h2_t = sb.tile([P, MC * NV2], f16)
            rt_t = sb.tile([32, R2 * OUT_C], f16)
            rw_t = sb.tile([P, OUT_W], f16)
            rw4_t = sb.tile([P, 4 * OUT_W], f16)
            av2_t = sb.tile([P, 2 * OUT_W], f16)

            # ---- input streams (big descriptors, balanced queues) --------
            # sync:   xs, w1a, em, w2b   + writes: pure0, tail1
            # scalar: xh, cb, w1b, wh, w2a + writes: tail0, pure1
            nc.sync.dma_start(x_t[:], xs_d.ap())
            nc.scalar.dma_start(xh_t[:], xh_d.ap())
            nc.scalar.dma_start(cb_t[:], cb_d.ap())
            nc.sync.dma_start(w1a_t[:], w1a_d.ap())
            nc.scalar.dma_start(w1b_t[:], w1b_d.ap())
            nc.scalar.dma_start(wh_t[:], wh_d.ap())
            nc.sync.dma_start(em_t[:], em_d.ap())
            nc.scalar.dma_start(w2a_t[:], w2a_d.ap())
            nc.sync.dma_start(w2b_t[:], w2b_d.ap())

            # h1 pads must be zero; activation only writes valid 32-col spans.
            nc.vector.memset(h1_t[:], 0.0)
            nc.gpsimd.memset(wz_t[:], 0.0)

            # bias views (f32 values stored as f16 pairs in the combo tile)
            def b1_ap(mc):
                return cb_t[:, 2 * mc: 2 * mc + 2].bitcast(f32)

            def b2_ap(mc):
                return cb_t[:, 4 + 2 * mc: 6 + 2 * mc].bitcast(f32)

            def br_ap(ps):
                return cb_t[ps:ps + 48, 8:10].bitcast(f32)

            # ---- PE warm-up: release the HAM clock gate during DMA wait --
            ps_warm = psp.tile([P, N1], f32, tag="cva", name="warm")
            for _ in range(16):
                nc.tensor.matmul(ps_warm[:, :], lhsT=wz_t[:, 0:P],
                                 rhs=wz_t[:, :], start=True, stop=True)

            # ---- conv1: 576 -> 256 over 7 rows ---------------------------
            ps1s = [psp.tile([P, N1], f32, tag="cva", name="ps1a"),
                    psp.tile([P, N1], f32, tag="cvb", name="ps1b")]

            def c1mm(w_t, col, tap, start, stop, kpart=P):
                ky, kx = tap // 3, tap % 3
                off = ky * W36 + kx + 1
                for mc in range(MC):
                    if kpart == P:
                        kc = (col // TB1) % KC1
                        lhs = w_t[:, col + mc * P: col + mc * P + P]
                        r = x_t[:, kc * XBLK + off: kc * XBLK + off + N1]
                    else:
                        lhs = w_t[0:64, col + mc * P: col + mc * P + P]
                        r = xh_t[0:64, off: off + N1]
                    nc.tensor.matmul(ps1s[mc][:, :], lhsT=lhs, rhs=r,
                                     start=start, stop=stop)

            first = True
            for tl in range(W1A):          # taps 0..4
                for kc in range(KC1):
                    c1mm(w1a_t, (tl * KC1 + kc) * TB1, tl, first, False)
                    first = False
            for tl in range(W1B):          # taps 5..8
                for kc in range(KC1):
                    c1mm(w1b_t, (tl * KC1 + kc) * TB1, 5 + tl, False, False)
            for tap in range(9):           # K=64 chunk, input ch 512..575
                c1mm(wh_t, tap * TB1, tap, False, tap == 8, kpart=64)

            for mc in range(MC):
                # ReLU(x + b) into the valid 32-wide spans of padded h1 rows
                ps1 = ps1s[mc]
                src = bass.AP(ps1.tensor, ps1.offset,
                              [[N1, P], [W36, R1], [1, 32]])
                dstb = h1_t[:, :]
                dst = bass.AP(dstb.tensor, dstb.offset + mc * H1BLK + 2,
                              [[MC * H1BLK + H1SLACK, P], [W36, R1], [1, 32]])
                nc.scalar.activation(dst, src, Relu, bias=b1_ap(mc))

            # zero h1 rows outside the global image (cores 0 and 7): per-row
            # mask broadcast over the 36 cols of each row
            for mc in range(MC):
                h1b = h1_t[:, :]
                cbb = cb_t[:, :]
                mask = bass.AP(cbb.tensor, cbb.offset + 58,
                               [[CB, P], [1, R1], [0, W36]])
                h1ap3 = bass.AP(h1b.tensor, h1b.offset + mc * H1BLK,
                                [[MC * H1BLK + H1SLACK, P], [W36, R1], [1, W36]])
                nc.vector.tensor_mul(h1ap3, h1ap3, mask)

            # ---- conv2: 256 -> 256 over 5 rows ---------------------------
            ps2s = [psp.tile([P, N2], f32, tag="cva", name="ps2a"),
                    psp.tile([P, N2], f32, tag="cvb", name="ps2b")]

            def c2mm(w_t, tl, tap, start, stop):
                ky, kx = tap // 3, tap % 3
                off = ky * W36 + kx + 1
                for kc in range(KC2):
                    for mc in range(MC):
                        nc.tensor.matmul(
                            ps2s[mc][:, :],
                            lhsT=w_t[:, (tl * KC2 + kc) * MC * P + mc * P:
                                     (tl * KC2 + kc) * MC * P + mc * P + P],
                            rhs=h1_t[:, kc * H1BLK + off: kc * H1BLK + off + N2],
                            start=start and kc == 0,
                            stop=stop and kc == KC2 - 1,
                        )

            for tl in range(W2A):
                c2mm(w2a_t, tl, tl, tl == 0, False)
            for tl in range(W2B):
                c2mm(w2b_t, tl, 6 + tl, False, tl == W2B - 1)

            # h2 = relu(conv2 + b2) on the DVE (scalar stays free)
            for mc in range(MC):
                ps2 = ps2s[mc]
                src2 = bass.AP(ps2.tensor, ps2.offset,
                               [[N2, P], [W36, R2], [1, 32]])
                h2b = h2_t[:, :]
                dst2 = bass.AP(h2b.tensor, h2b.offset + mc * NV2,
                               [[MC * NV2, P], [W36, R2], [1, 32]])
                nc.vector.tensor_scalar(dst2, src2, b2_ap(mc), 0.0,
                                        Alu.add, Alu.max)

            # ---- 1x1 conv 256 -> 24 into (w, (h, c)) h-major -------------
            psr = psp.tile([32, R2 * OUT_C], f32, tag="psr")
            for h in range(R2):
                for kc in range(KC2):
                    nc.tensor.matmul(
                        psr[:, h * OUT_C:(h + 1) * OUT_C],
                        lhsT=h2_t[:, kc * NV2 + h * W36: kc * NV2 + h * W36 + 32],
                        rhs=cb_t[:, 10 + kc * OUT_C: 10 + (kc + 1) * OUT_C],
                        start=(kc == 0), stop=(kc == KC2 - 1),
                    )
            # h-major layout means rt is a plain copy of psr
            nc.vector.tensor_copy(rt_t[:, :], psr[:, :])

            # ---- chunked W expansion + H replication + writes ------------
            # chunk 0: owned rows 0,1 -> partitions 0..47; chunk 1: rows 2,3
            # -> partitions 64..111 (PE out base must be 0/32/64).  HBM row
            # offset for (h, c) is RUN*OUT_W*(24h + c).
            psw = psp.tile([P, OUT_W], f32, tag="psw")
            psa = psp.tile([P, OUT_W], f32, tag="psa")
            for ch in range(2):
                ps = 64 * ch           # sbuf/psum partition base
                pd = 48 * ch           # HBM row-block base
                lhs_pure = rt_t[:, 48 * ch: 48 * ch + 48]
                lhs_next = rt_t[:, 48 * ch + OUT_C: 48 * ch + OUT_C + 48]
                for j in range(2):
                    nc.tensor.matmul(psw[ps:ps + 48, j * 512:(j + 1) * 512],
                                     lhsT=lhs_pure,
                                     rhs=em_t[:, j * 512:(j + 1) * 512],
                                     start=True, stop=True)
                for j in range(2):
                    nc.tensor.matmul(psa[ps:ps + 48, j * 512:(j + 1) * 512],
                                     lhsT=lhs_pure,
                                     rhs=em_t[:, j * 512:(j + 1) * 512],
                                     start=True, stop=False)
                    nc.tensor.matmul(psa[ps:ps + 48, j * 512:(j + 1) * 512],
                                     lhsT=lhs_next,
                                     rhs=em_t[:, j * 512:(j + 1) * 512],
                                     start=False, stop=True)
                # pure rows + bias on scalar; 4x replicate on DVE
                nc.scalar.activation(rw_t[ps:ps + 48, :], psw[ps:ps + 48, :],
                                     Ident, bias=br_ap(ps))
                rwb = rw_t[:, :]
                rep_src = bass.AP(rwb.tensor, rwb.offset + ps * OUT_W,
                                  [[OUT_W, 48], [0, 4], [1, OUT_W]])
                r4b = rw4_t[:, :]
                rep_dst = bass.AP(r4b.tensor, r4b.offset + ps * 4 * OUT_W,
                                  [[4 * OUT_W, 48], [OUT_W, 4], [1, OUT_W]])
                nc.vector.tensor_copy(rep_dst, rep_src)
                # tail buffer: row 20 (pure) + row 21 (avg = 0.5*sum + bias)
                nc.vector.tensor_copy(av2_t[ps:ps + 48, 0:OUT_W],
                                      rw_t[ps:ps + 48, :])
                nc.vector.tensor_scalar(av2_t[ps:ps + 48, OUT_W:2 * OUT_W],
                                        psa[ps:ps + 48, :], 0.5, br_ap(ps),
                                        Alu.mult, Alu.add)
                a2b = av2_t[:, :]
                srcp = bass.AP(r4b.tensor, r4b.offset + ps * 4 * OUT_W,
                               [[4 * OUT_W, 48], [0, 5], [1, 4 * OUT_W]])
                dstp = bass.AP(out_d, pd * RUN * OUT_W,
                               [[RUN * OUT_W, 48], [4 * OUT_W, 5], [1, 4 * OUT_W]])
                srct = bass.AP(a2b.tensor, a2b.offset + ps * 2 * OUT_W,
                               [[2 * OUT_W, 48], [1, 2 * OUT_W]])
                dstt = bass.AP(out_d, pd * RUN * OUT_W + 20 * OUT_W,
                               [[RUN * OUT_W, 48], [1, 2 * OUT_W]])
                engp = nc.sync if ch == 0 else nc.scalar
                engt = nc.scalar if ch == 0 else nc.sync
                engp.dma_start(dstp, srcp)
                engt.dma_start(dstt, srct)

    nc.compile()
    return nc


def _pack_inputs(x, w1, b1, w2, b2, wr, br):
    x = np.asarray(x, np.float32)
    w1 = np.asarray(w1, np.float32)
    w2 = np.asarray(w2, np.float32)
    wr = np.asarray(wr, np.float32)
    b1 = np.asarray(b1, np.float32)
    b2 = np.asarray(b2, np.float32)
    br = np.asarray(br, np.float32)

    xv = x[0]  # (576, 32, 32)
    xp = np.zeros((NCORES, P, KC1, RX, W36), np.float16)
    xhp = np.zeros((NCORES, 64, RX, W36), np.float16)
    for k in range(NCORES):
        for r in range(RX):
            g = 4 * k - 2 + r
            if 0 <= g < H:
                blkv = xv[:, g, :]  # (576, 32)
                xp[k, :, :, r, 2:34] = blkv[:512].reshape(KC1, P, W).transpose(1, 0, 2)
                xhp[k, :, r, 2:34] = blkv[512:]
    xp = xp.reshape(NCORES, P, KC1 * XBLK)
    xp = np.concatenate([xp, np.zeros((NCORES, P, XSLACK), np.float16)], axis=2)
    xhp = xhp.reshape(NCORES, 64, XBLK)
    xhp = np.concatenate([xhp, np.zeros((NCORES, 64, XSLACK), np.float16)],
                         axis=2)

    # w1 full chunks: [p, tap, kc, mc, m] = w1[mc*128+m, kc*128+p, ky, kx]
    w1v = w1.transpose(2, 3, 1, 0).reshape(9, IN_C, MID_C)  # (tap, ci, co)
    w1p = (w1v[:, :512, :].reshape(9, KC1, P, MC, P)
           .transpose(2, 0, 1, 3, 4).reshape(P, 9 * KC1 * MC * P))
    w1ap = np.ascontiguousarray(w1p[:, :5 * KC1 * MC * P], np.float16)
    w1bp = np.ascontiguousarray(w1p[:, 5 * KC1 * MC * P:], np.float16)
    # w1 K=64 chunk: [p, tap, mc, m] = w1[mc*128+m, 512+p, ky, kx]
    whp = (w1v[:, 512:, :].reshape(9, 64, MC, P)
           .transpose(1, 0, 2, 3).reshape(64, 9 * MC * P))
    whp = np.ascontiguousarray(whp, np.float16)

    # w2: [p, tap, kc, mc, m]
    w2v = w2.transpose(2, 3, 1, 0).reshape(9, MID_C, MID_C)
    w2p = (w2v.reshape(9, KC2, P, MC, P).transpose(2, 0, 1, 3, 4)
           .reshape(P, 9 * KC2 * MC * P))
    w2ap = np.ascontiguousarray(w2p[:, :6 * KC2 * MC * P], np.float16)
    w2bp = np.ascontiguousarray(w2p[:, 6 * KC2 * MC * P:], np.float16)

    wrp = wr.T.reshape(KC2, P, OUT_C).transpose(1, 0, 2).reshape(P, KC2 * OUT_C)
    # combo tile: f32 biases as f16 pairs | wr | h1 row mask
    bbf = np.zeros((P, 5), np.float32)
    bbf[:, 0:2] = b1.reshape(MC, P).T
    bbf[:, 2:4] = b2.reshape(MC, P).T
    # expansion-chunk bias: partitions 0..47 and 64..111 hold br[p % 24]
    bbf[0:48, 4] = np.tile(br, 2)
    bbf[64:112, 4] = np.tile(br, 2)
    cbp = np.zeros((NCORES, P, CB), np.float16)
    cbp[:, :, 0:10] = bbf.view(np.float16)[None]
    cbp[:, :, 10:58] = wrp.astype(np.float16)[None]
    for k in range(NCORES):
        for r in range(R1):
            if 0 <= 4 * k - 1 + r < H:
                cbp[k, :, 58 + r] = 1.0

    em = (np.arange(OUT_W) // 32 == np.arange(32)[:, None]).astype(np.float16)

    shared = dict(w1a=w1ap, w1b=w1bp, whp=whp, w2a=w2ap, w2b=w2bp, emp=em)
    in_maps = []
    for k in range(NCORES):
        m = dict(shared)
        m["xs"] = np.ascontiguousarray(xp[k])
        m["xh"] = np.ascontiguousarray(xhp[k])
        m["cbp"] = np.ascontiguousarray(cbp[k])
        in_maps.append(m)
    return in_maps


def kernel(x, w1, b1, w2, b2, wr, br):
    from concourse.bass_utils import run_bass_kernel_spmd

    if "nc" not in _prog_cache:
        _prog_cache["nc"] = _build_program()
    nc = _prog_cache["nc"]

    in_maps = _pack_inputs(x, w1, b1, w2, b2, wr, br)
    res = run_bass_kernel_spmd(nc, in_maps, list(range(NCORES)))

    _, t = _h_runs()
    out = np.empty((1, OUT_C, OUT_H, OUT_W), np.float32)
    for k in range(NCORES):
        # (4*24*22, 1024) rows ordered (h, c, run) h-major
        buf = res.results[k]["outb"].astype(np.float32)
        buf = buf.reshape(4, OUT_C, RUN, OUT_W)
        for hl in range(4):
            h = 4 * k + hl
            n = t[h + 1] - t[h]
            if h < H - 1:
                out[0, :, t[h]:t[h] + n - 1, :] = buf[hl, :, :n - 1, :]
                out[0, :, t[h] + n - 1, :] = buf[hl, :, RUN - 1, :]
            else:
                out[0, :, t[h]:t[h] + n, :] = buf[hl, :, :n, :]
    return out


# revision 17
# speedup vs baseline: 1.1459x; 1.1459x over previous
"""Trainium2 Bass kernel for nn_ConvProjector (conv3x3 -> ReLU -> conv3x3 -> ReLU
-> adaptive-avg-pool upsample 32x32 -> 687x1024 -> 1x1 conv 256->24 + bias).

Strategy (v4):
  * Pool and 1x1 conv commute: reduce 256->24 channels at 32x32 first, then
    upsample only 24 channels (expansion matmul for W, stride-0 DMA
    replication for H, averaged boundary rows from a second matmul).
  * Sharding: core k owns input rows 4k..4k+3 (+halos); no collectives.
  * Perf model (measured): each DMA queue's descriptor generator sustains
    ~60M desc/s, so queue rate = desc_size * 60M/s up to ~360 GB/s.  Three
    queues (sync/scalar HWDGE + gpsimd SWDGE) are used for both the read
    and write streams; w1/w2 are packed as tap-triple blocks (6KB
    descriptors) so conv matmuls stream as each triple lands; the output's
    21 replicated rows are written 7 rows per queue.
  * PE warm-up matmuls are chained on the x DMA so the HAM clock-gate
    opens right as the first weight block lands (no idle re-cool).
  * Back-end is full-width (96+ partitions): psr -> rt copy; rts = rt rows
    h + h+1 (DVE add) halves the averaged-expansion matmuls; rw/av biases
    applied on the DVE.
Output is assembled on the host from the per-core (4, 24, 22, 1024) buffers.
"""
import sys

if '/opt/trn_rl_repo' not in sys.path:
    sys.path.insert(0, '/opt/trn_rl_repo')

import numpy as np

IN_C, MID_C, OUT_C = 576, 256, 24
H = W = 32
OUT_H, OUT_W = 687, 1024
NCORES = 8
P = 128
KC1 = 4           # full 128-channel input chunks for conv1 (plus one 64 chunk)
KC2 = 2           # 256/128 chunks for conv2 / 1x1
MC = 2            # 256/128 output-channel chunks for conv1/conv2
W36 = 36          # padded row width (2 zero cols each side)
RX, R1, R2 = 9, 7, 5          # x rows / h1 rows / h2 (=r) rows per core
XBLK = RX * W36               # 324  per-kc x block
XSLACK = 16                   # rhs overrun slack so N can pad to 256
N1 = 256                      # conv1 matmul N (covers the 7 h1 rows)
H1BLK = R1 * W36              # 252  per-mc h1 block
H1SLACK = 80
N2 = 180                      # conv2 matmul N (covers the 5 h2 rows)
NV2 = 176                     # valid h2 flat span per mc
RUN = 22                      # output rows per owned input row in core buffer
CB = 66                       # combo cols: 10 bias(f16 pairs) + 48 wr + 7 mask + pad
TB1 = MC * P                  # 256: conv1 per-(tap,kc) lhsT block
TB2 = KC2 * MC * P            # 512: conv2 per-tap block

_prog_cache = {}


def _h_runs():
    i = np.arange(OUT_H)
    s = (i * H) // OUT_H
    t = np.searchsorted(s, np.arange(H + 1), side='left')
    return s, t


def _build_program():
    import concourse.bass as bass
    import concourse.bacc as bacc
    import concourse.mybir as mybir
    from concourse.tile import TileContext

    f32 = mybir.dt.float32
    f16 = mybir.dt.float16
    Alu = mybir.AluOpType
    nc = bacc.Bacc("TRN2", target_bir_lowering=False, debug=False,
                   num_devices=NCORES)

    xs_d = nc.dram_tensor("xs", [P, KC1 * XBLK + XSLACK], f16, kind="ExternalInput")
    xh_d = nc.dram_tensor("xh", [64, XBLK + XSLACK], f16, kind="ExternalInput")
    w1_ds = [nc.dram_tensor(f"w1t{i}", [P, 3 * KC1 * TB1], f16,
                            kind="ExternalInput") for i in range(3)]
    wh_d = nc.dram_tensor("whp", [64, 9 * TB1], f16, kind="ExternalInput")
    w2_ds = [nc.dram_tensor(f"w2t{i}", [P, 3 * TB2], f16,
                            kind="ExternalInput") for i in range(3)]
    cb_d = nc.dram_tensor("cbp", [P, CB], f16, kind="ExternalInput")
    em_d = nc.dram_tensor("emp", [32, OUT_W], f16, kind="ExternalInput")
    out_d = nc.dram_tensor("outb", [4 * OUT_C * RUN, OUT_W], f16,
                           kind="ExternalOutput")

    Relu = mybir.ActivationFunctionType.Relu

    with TileContext(nc) as tc:
        with (
            tc.tile_pool(name="sb", bufs=1) as sb,
            tc.tile_pool(name="ps", bufs=1, space="PSUM") as psp,
        ):
            wz_t = sb.tile([P, P], f16)
            x_t = sb.tile([P, KC1 * XBLK + XSLACK], f16)
            xh_t = sb.tile([64, XBLK + XSLACK], f16)
            w1_ts = [sb.tile([P, 3 * KC1 * TB1], f16, name=f"w1t{i}")
                     for i in range(3)]
            wh_t = sb.tile([64, 9 * TB1], f16)
            w2_ts = [sb.tile([P, 3 * TB2], f16, name=f"w2t{i}")
                     for i in range(3)]
            cb_t = sb.tile([P, CB], f16)
            em_t = sb.tile([32, OUT_W], f16)
            h1_t = sb.tile([P, MC * H1BLK + H1SLACK], f16)
            h2_t = sb.tile([P, MC * NV2], f16)
            rt_t = sb.tile([32, R2 * OUT_C], f16)
            rts_t = sb.tile([32, 4 * OUT_C], f16)
            rw_t = sb.tile([96, OUT_W], f16)
            rw7_t = sb.tile([96, 7 * OUT_W], f16)
            av_t = sb.tile([96, OUT_W], f16)

            # ---- read streams: 3 queues, tap-triple granularity ----------
            # sync:   xs, w1T0, w2T2            (~1.51 MB)
            # scalar: xh, cb, w1T1, wh, w2T0    (~1.53 MB)
            # gpsimd: w1T2, em, w2T1            (~1.24 MB)
            nc.sync.dma_start(x_t[:], xs_d.ap())
            nc.scalar.dma_start(xh_t[:], xh_d.ap())
            nc.scalar.dma_start(cb_t[:], cb_d.ap())
            nc.sync.dma_start(w1_ts[0][:], w1_ds[0].ap())
            nc.scalar.dma_start(w1_ts[1][:], w1_ds[1].ap())
            nc.gpsimd.dma_start(w1_ts[2][:], w1_ds[2].ap())
            nc.scalar.dma_start(wh_t[:], wh_d.ap())
            nc.gpsimd.dma_start(em_t[:], em_d.ap())
            nc.scalar.dma_start(w2_ts[0][:], w2_ds[0].ap())
            nc.gpsimd.dma_start(w2_ts[1][:], w2_ds[1].ap())
            nc.sync.dma_start(w2_ts[2][:], w2_ds[2].ap())

            # h1 pads must be zero; activation only writes valid 32-col spans.
            nc.vector.memset(h1_t[:], 0.0)
            nc.vector.memset(wz_t[:], 0.0)

            def b1_ap(mc):
                return cb_t[:, 2 * mc: 2 * mc + 2].bitcast(f32)

            def b2_ap(mc):
                return cb_t[:, 4 + 2 * mc: 6 + 2 * mc].bitcast(f32)

            br96 = cb_t[0:96, 8:10].bitcast(f32)

            # ---- PE warm-up chained on the x DMA: opens the HAM clock
            # gate right before the first weight triple lands -------------
            ps_warm = psp.tile([P, N1], f32, tag="cva", name="warm")
            for _ in range(10):
                nc.tensor.matmul(ps_warm[:, :], lhsT=wz_t[:, :],
                                 rhs=x_t[:, 0:N1], start=True, stop=True)

            # ---- conv1: 576 -> 256 over 7 rows, triple-streamed ----------
            ps1s = [psp.tile([P, N1], f32, tag="cva", name="ps1a"),
                    psp.tile([P, N1], f32, tag="cvb", name="ps1b")]

            def c1mm(w_t, col, tap, start, stop, half=False):
                ky, kx = tap // 3, tap % 3
                off = ky * W36 + kx + 1
                for mc in range(MC):
                    if half:
                        lhs = w_t[0:64, col + mc * P: col + mc * P + P]
                        r = xh_t[0:64, off: off + N1]
                    else:
                        kc = (col // TB1) % KC1
                        lhs = w_t[:, col + mc * P: col + mc * P + P]
                        r = x_t[:, kc * XBLK + off: kc * XBLK + off + N1]
                    nc.tensor.matmul(ps1s[mc][:, :], lhsT=lhs, rhs=r,
                                     start=start, stop=stop)

            first = True
            for tr in range(3):            # triples in arrival order
                for tl in range(3):
                    for kc in range(KC1):
                        c1mm(w1_ts[tr], (tl * KC1 + kc) * TB1, tr * 3 + tl,
                             first, False)
                        first = False
            for tap in range(9):           # K=64 chunk, input ch 512..575
                c1mm(wh_t, tap * TB1, tap, False, tap == 8, half=True)

            for mc in range(MC):
                # ReLU(x + b) into the valid 32-wide spans of padded h1 rows
                ps1 = ps1s[mc]
                src = bass.AP(ps1.tensor, ps1.offset,
                              [[N1, P], [W36, R1], [1, 32]])
                dstb = h1_t[:, :]
                dst = bass.AP(dstb.tensor, dstb.offset + mc * H1BLK + 2,
                              [[MC * H1BLK + H1SLACK, P], [W36, R1], [1, 32]])
                nc.scalar.activation(dst, src, Relu, bias=b1_ap(mc))

            # zero h1 rows outside the global image (cores 0 and 7)
            for mc in range(MC):
                h1b = h1_t[:, :]
                cbb = cb_t[:, :]
                mask = bass.AP(cbb.tensor, cbb.offset + 58,
                               [[CB, P], [1, R1], [0, W36]])
                h1ap3 = bass.AP(h1b.tensor, h1b.offset + mc * H1BLK,
                                [[MC * H1BLK + H1SLACK, P], [W36, R1], [1, W36]])
                nc.vector.tensor_mul(h1ap3, h1ap3, mask)

            # ---- conv2: 256 -> 256 over 5 rows, triple-streamed ----------
            ps2s = [psp.tile([P, N2], f32, tag="cva", name="ps2a"),
                    psp.tile([P, N2], f32, tag="cvb", name="ps2b")]
            for tr in range(3):
                for tl in range(3):
                    tap = tr * 3 + tl
                    ky, kx = tap // 3, tap % 3
                    off = ky * W36 + kx + 1
                    for kc in range(KC2):
                        for mc in range(MC):
                            nc.tensor.matmul(
                                ps2s[mc][:, :],
                                lhsT=w2_ts[tr][:, (tl * KC2 + kc) * MC * P
                                               + mc * P:
                                               (tl * KC2 + kc) * MC * P
                                               + mc * P + P],
                                rhs=h1_t[:, kc * H1BLK + off:
                                         kc * H1BLK + off + N2],
                                start=(tap == 0 and kc == 0),
                                stop=(tap == 8 and kc == KC2 - 1),
                            )

            # h2 = relu(conv2 + b2) on the DVE
            for mc in range(MC):
                ps2 = ps2s[mc]
                src2 = bass.AP(ps2.tensor, ps2.offset,
                               [[N2, P], [W36, R2], [1, 32]])
                h2b = h2_t[:, :]
                dst2 = bass.AP(h2b.tensor, h2b.offset + mc * NV2,
                               [[MC * NV2, P], [W36, R2], [1, 32]])
                nc.vector.tensor_scalar(dst2, src2, b2_ap(mc), 0.0,
                                        Alu.add, Alu.max)

            # ---- 1x1 conv 256 -> 24 into (w, (h, c)) h-major -------------
            psr = psp.tile([32, R2 * OUT_C], f32, tag="psr")
            for h in range(R2):
                for kc in range(KC2):
                    nc.tensor.matmul(
                        psr[:, h * OUT_C:(h + 1) * OUT_C],
                        lhsT=h2_t[:, kc * NV2 + h * W36: kc * NV2 + h * W36 + 32],
                        rhs=cb_t[:, 10 + kc * OUT_C: 10 + (kc + 1) * OUT_C],
                        start=(kc == 0), stop=(kc == KC2 - 1),
                    )
            nc.vector.tensor_copy(rt_t[:, :], psr[:, :])
            # rts[w, 24h+c] = r[h] + r[h+1] for the averaged boundary rows
            nc.vector.tensor_tensor(rts_t[:, :], rt_t[:, 0:4 * OUT_C],
                                    rt_t[:, OUT_C:5 * OUT_C], Alu.add)

            # ---- W expansion (full width) --------------------------------
            psw = psp.tile([96, OUT_W], f32, tag="psw")
            psa = psp.tile([96, OUT_W], f32, tag="psa")
            for j in range(2):
                nc.tensor.matmul(psw[:, j * 512:(j + 1) * 512],
                                 lhsT=rt_t[:, 0:96],
                                 rhs=em_t[:, j * 512:(j + 1) * 512],
                                 start=True, stop=True)
            for j in range(2):
                nc.tensor.matmul(psa[:, j * 512:(j + 1) * 512],
                                 lhsT=rts_t[:, :],
                                 rhs=em_t[:, j * 512:(j + 1) * 512],
                                 start=True, stop=True)
            # rw = psw + br ; av = 0.5*psa + br   (both on the DVE)
            nc.vector.tensor_scalar(rw_t[:, :], psw[:, :], br96, None, Alu.add)
            nc.vector.tensor_scalar(av_t[:, :], psa[:, :], 0.5, br96,
                                    Alu.mult, Alu.add)

            # ---- H replication writes (hybrid): stride-0-source writes cap
            # ~230 GB/s total (engine M2S stalls on repeated reads), so rows
            # 0-13 go stride-0 on the two HWDGE queues while rows 14-20 are
            # DVE-materialized into rw7 and written contiguously (16KB
            # descriptors, ~346 GB/s) on the gpsimd queue + averaged row --
            rwb = rw_t[:, :]
            avb = av_t[:, :]
            r7b = rw7_t[:, :]
            nc.vector.tensor_copy(
                bass.AP(r7b.tensor, r7b.offset,
                        [[7 * OUT_W, 96], [OUT_W, 7], [1, OUT_W]]),
                bass.AP(rwb.tensor, rwb.offset,
                        [[OUT_W, 96], [0, 7], [1, OUT_W]]))
            s7 = bass.AP(rwb.tensor, rwb.offset, [[OUT_W, 96], [0, 7], [1, OUT_W]])
            for qi, eng in enumerate((nc.sync, nc.scalar)):
                dst = bass.AP(out_d, qi * 7 * OUT_W,
                              [[RUN * OUT_W, 96], [OUT_W, 7], [1, OUT_W]])
                eng.dma_start(dst, s7)
            dst7 = bass.AP(out_d, 14 * OUT_W,
                           [[RUN * OUT_W, 96], [1, 7 * OUT_W]])
            nc.gpsimd.dma_start(dst7, bass.AP(r7b.tensor, r7b.offset,
                                              [[7 * OUT_W, 96], [1, 7 * OUT_W]]))
            srca = bass.AP(avb.tensor, avb.offset, [[OUT_W, 96], [1, OUT_W]])
            dsta = bass.AP(out_d, 21 * OUT_W, [[RUN * OUT_W, 96], [1, OUT_W]])
            nc.gpsimd.dma_start(dsta, srca)

    nc.compile()
    return nc


def _pack_inputs(x, w1, b1, w2, b2, wr, br):
    x = np.asarray(x, np.float32)
    w1 = np.asarray(w1, np.float32)
    w2 = np.asarray(w2, np.float32)
    wr = np.asarray(wr, np.float32)
    b1 = np.asarray(b1, np.float32)
    b2 = np.asarray(b2, np.float32)
    br = np.asarray(br, np.float32)

    xv = x[0]  # (576, 32, 32)
    xp = np.zeros((NCORES, P, KC1, RX, W36), np.float16)
    xhp = np.zeros((NCORES, 64, RX, W36), np.float16)
    for k in range(NCORES):
        for r in range(RX):
            g = 4 * k - 2 + r
            if 0 <= g < H:
                blkv = xv[:, g, :]  # (576, 32)
                xp[k, :, :, r, 2:34] = blkv[:512].reshape(KC1, P, W).transpose(1, 0, 2)
                xhp[k, :, r, 2:34] = blkv[512:]
    xp = xp.reshape(NCORES, P, KC1 * XBLK)
    xp = np.concatenate([xp, np.zeros((NCORES, P, XSLACK), np.float16)], axis=2)
    xhp = xhp.reshape(NCORES, 64, XBLK)
    xhp = np.concatenate([xhp, np.zeros((NCORES, 64, XSLACK), np.float16)],
                         axis=2)

    # w1 full chunks: [p, tap, kc, mc, m] = w1[mc*128+m, kc*128+p, ky, kx]
    w1v = w1.transpose(2, 3, 1, 0).reshape(9, IN_C, MID_C)  # (tap, ci, co)
    w1p = (w1v[:, :512, :].reshape(9, KC1, P, MC, P)
           .transpose(2, 0, 1, 3, 4).reshape(P, 9 * KC1 * TB1))
    w1ts = [np.ascontiguousarray(w1p[:, i * 3 * KC1 * TB1:(i + 1) * 3 * KC1 * TB1],
                                 np.float16) for i in range(3)]
    whp = (w1v[:, 512:, :].reshape(9, 64, MC, P)
           .transpose(1, 0, 2, 3).reshape(64, 9 * TB1))
    whp = np.ascontiguousarray(whp, np.float16)

    w2v = w2.transpose(2, 3, 1, 0).reshape(9, MID_C, MID_C)
    w2p = (w2v.reshape(9, KC2, P, MC, P).transpose(2, 0, 1, 3, 4)
           .reshape(P, 9 * TB2))
    w2ts = [np.ascontiguousarray(w2p[:, i * 3 * TB2:(i + 1) * 3 * TB2],
                                 np.float16) for i in range(3)]

    wrp = wr.T.reshape(KC2, P, OUT_C).transpose(1, 0, 2).reshape(P, KC2 * OUT_C)
    bbf = np.zeros((P, 5), np.float32)
    bbf[:, 0:2] = b1.reshape(MC, P).T
    bbf[:, 2:4] = b2.reshape(MC, P).T
    bbf[:96, 4] = np.tile(br, 4)          # br[p % 24] for p = 24h + c
    cbp = np.zeros((NCORES, P, CB), np.float16)
    cbp[:, :, 0:10] = bbf.view(np.float16)[None]
    cbp[:, :, 10:58] = wrp.astype(np.float16)[None]
    for k in range(NCORES):
        for r in range(R1):
            if 0 <= 4 * k - 1 + r < H:
                cbp[k, :, 58 + r] = 1.0

    em = (np.arange(OUT_W) // 32 == np.arange(32)[:, None]).astype(np.float16)

    shared = dict(whp=whp, emp=em)
    for i in range(3):
        shared[f"w1t{i}"] = w1ts[i]
        shared[f"w2t{i}"] = w2ts[i]
    in_maps = []
    for k in range(NCORES):
        m = dict(shared)
        m["xs"] = np.ascontiguousarray(xp[k])
        m["xh"] = np.ascontiguousarray(xhp[k])
        m["cbp"] = np.ascontiguousarray(cbp[k])
        in_maps.append(m)
    return in_maps


def kernel(x, w1, b1, w2, b2, wr, br):
    from concourse.bass_utils import run_bass_kernel_spmd

    if "nc" not in _prog_cache:
        _prog_cache["nc"] = _build_program()
    nc = _prog_cache["nc"]

    in_maps = _pack_inputs(x, w1, b1, w2, b2, wr, br)
    res = run_bass_kernel_spmd(nc, in_maps, list(range(NCORES)))

    _, t = _h_runs()
    out = np.empty((1, OUT_C, OUT_H, OUT_W), np.float32)
    for k in range(NCORES):
        # (4*24*22, 1024) rows ordered (h, c, run) h-major
        buf = res.results[k]["outb"].astype(np.float32)
        buf = buf.reshape(4, OUT_C, RUN, OUT_W)
        for hl in range(4):
            h = 4 * k + hl
            n = t[h + 1] - t[h]
            if h < H - 1:
                out[0, :, t[h]:t[h] + n - 1, :] = buf[hl, :, :n - 1, :]
                out[0, :, t[h] + n - 1, :] = buf[hl, :, RUN - 1, :]
            else:
                out[0, :, t[h]:t[h] + n, :] = buf[hl, :, :n, :]
    return out


# revision 24
# speedup vs baseline: 1.1567x; 1.0094x over previous
"""Trainium2 Bass kernel for nn_ConvProjector (conv3x3 -> ReLU -> conv3x3 -> ReLU
-> adaptive-avg-pool upsample 32x32 -> 687x1024 -> 1x1 conv 256->24 + bias).

Strategy (v4):
  * Pool and 1x1 conv commute: reduce 256->24 channels at 32x32 first, then
    upsample only 24 channels (expansion matmul for W, stride-0 DMA
    replication for H, averaged boundary rows from a second matmul).
  * Sharding: core k owns input rows 4k..4k+3 (+halos); no collectives.
  * Perf model (measured): each DMA queue's descriptor generator sustains
    ~60M desc/s, so queue rate = desc_size * 60M/s up to ~360 GB/s.  Three
    queues (sync/scalar HWDGE + gpsimd SWDGE) are used for both the read
    and write streams; w1/w2 are packed as tap-triple blocks (6KB
    descriptors) so conv matmuls stream as each triple lands; the output's
    21 replicated rows are written 7 rows per queue.
  * PE warm-up matmuls are chained on the x DMA so the HAM clock-gate
    opens right as the first weight block lands (no idle re-cool).
  * Back-end is full-width (96+ partitions): psr -> rt copy; rts = rt rows
    h + h+1 (DVE add) halves the averaged-expansion matmuls; rw/av biases
    applied on the DVE.
Output is assembled on the host from the per-core (4, 24, 22, 1024) buffers.
"""
import sys

if '/opt/trn_rl_repo' not in sys.path:
    sys.path.insert(0, '/opt/trn_rl_repo')

import numpy as np

IN_C, MID_C, OUT_C = 576, 256, 24
H = W = 32
OUT_H, OUT_W = 687, 1024
NCORES = 8
P = 128
KC1 = 4           # full 128-channel input chunks for conv1 (plus one 64 chunk)
KC2 = 2           # 256/128 chunks for conv2 / 1x1
MC = 2            # 256/128 output-channel chunks for conv1/conv2
W36 = 36          # padded row width (2 zero cols each side)
RX, R1, R2 = 9, 7, 5          # x rows / h1 rows / h2 (=r) rows per core
XBLK = RX * W36               # 324  per-kc x block
XSLACK = 16                   # rhs overrun slack so N can pad to 256
N1 = 256                      # conv1 matmul N (covers the 7 h1 rows)
H1BLK = R1 * W36              # 252  per-mc h1 block
H1SLACK = 80
N2 = 180                      # conv2 matmul N (covers the 5 h2 rows)
NV2 = 176                     # valid h2 flat span per mc
RUN = 22                      # output rows per owned input row in core buffer
CB = 66                       # combo cols: 10 bias(f16 pairs) + 48 wr + 7 mask + pad
TB1 = MC * P                  # 256: conv1 per-(tap,kc) lhsT block
TB2 = KC2 * MC * P            # 512: conv2 per-tap block

_prog_cache = {}


def _h_runs():
    i = np.arange(OUT_H)
    s = (i * H) // OUT_H
    t = np.searchsorted(s, np.arange(H + 1), side='left')
    return s, t


def _build_program():
    import concourse.bass as bass
    import concourse.bacc as bacc
    import concourse.mybir as mybir
    from concourse.tile import TileContext

    f32 = mybir.dt.float32
    f16 = mybir.dt.float16
    Alu = mybir.AluOpType
    nc = bacc.Bacc("TRN2", target_bir_lowering=False, debug=False,
                   num_devices=NCORES)

    xs_d = nc.dram_tensor("xs", [P, KC1 * XBLK + XSLACK], f16, kind="ExternalInput")
    xh_d = nc.dram_tensor("xh", [64, XBLK + XSLACK], f16, kind="ExternalInput")
    w1_ds = [nc.dram_tensor(f"w1t{i}", [P, 3 * KC1 * TB1], f16,
                            kind="ExternalInput") for i in range(3)]
    wh_d = nc.dram_tensor("whp", [64, 9 * TB1], f16, kind="ExternalInput")
    w2_ds = [nc.dram_tensor(f"w2t{i}", [P, 3 * TB2], f16,
                            kind="ExternalInput") for i in range(3)]
    cb_d = nc.dram_tensor("cbp", [P, CB], f16, kind="ExternalInput")
    em_d = nc.dram_tensor("emp", [32, OUT_W], f16, kind="ExternalInput")
    out_d = nc.dram_tensor("outb", [4 * OUT_C * RUN, OUT_W], f16,
                           kind="ExternalOutput")

    Relu = mybir.ActivationFunctionType.Relu

    with TileContext(nc) as tc:
        with (
            tc.tile_pool(name="sb", bufs=1) as sb,
            tc.tile_pool(name="ps", bufs=1, space="PSUM") as psp,
        ):
            wz_t = sb.tile([P, 512], f16)
            x_t = sb.tile([P, KC1 * XBLK + XSLACK], f16)
            xh_t = sb.tile([64, XBLK + XSLACK], f16)
            w1_ts = [sb.tile([P, 3 * KC1 * TB1], f16, name=f"w1t{i}")
                     for i in range(3)]
            wh_t = sb.tile([64, 9 * TB1], f16)
            w2_ts = [sb.tile([P, 3 * TB2], f16, name=f"w2t{i}")
                     for i in range(3)]
            cb_t = sb.tile([P, CB], f16)
            em_t = sb.tile([32, OUT_W], f16)
            h1_t = sb.tile([P, MC * H1BLK + H1SLACK], f16)
            h2_t = sb.tile([P, MC * NV2], f16)
            rt_t = sb.tile([32, R2 * OUT_C], f16)
            rts_t = sb.tile([32, 4 * OUT_C], f16)
            rw_t = sb.tile([96, OUT_W], f16)
            rw7_t = sb.tile([96, 7 * OUT_W], f16)
            av_t = sb.tile([96, OUT_W], f16)

            # ---- read streams: 3 queues, tap-triple granularity ----------
            # sync:   xs, w1T0, w2T2            (~1.51 MB)
            # scalar: xh, cb, w1T1, wh, w2T0    (~1.53 MB)
            # gpsimd: w1T2, em, w2T1            (~1.24 MB)
            nc.sync.dma_start(x_t[:], xs_d.ap())
            nc.scalar.dma_start(xh_t[:], xh_d.ap())
            nc.scalar.dma_start(cb_t[:], cb_d.ap())
            nc.sync.dma_start(w1_ts[0][:], w1_ds[0].ap())
            nc.scalar.dma_start(w1_ts[1][:], w1_ds[1].ap())
            nc.gpsimd.dma_start(w1_ts[2][:], w1_ds[2].ap())
            nc.gpsimd.dma_start(wh_t[:], wh_d.ap())
            nc.gpsimd.dma_start(em_t[:], em_d.ap())
            nc.scalar.dma_start(w2_ts[0][:], w2_ds[0].ap())
            nc.gpsimd.dma_start(w2_ts[1][:], w2_ds[1].ap())
            nc.sync.dma_start(w2_ts[2][:], w2_ds[2].ap())

            # h1 pads must be zero; activation only writes valid 32-col spans.
            nc.vector.memset(h1_t[:], 0.0)
            nc.vector.memset(wz_t[:], 0.0)

            def b1_ap(mc):
                return cb_t[:, 2 * mc: 2 * mc + 2].bitcast(f32)

            def b2_ap(mc):
                return cb_t[:, 4 + 2 * mc: 6 + 2 * mc].bitcast(f32)

            br96 = cb_t[0:96, 8:10].bitcast(f32)

            # ---- PE warm-up burst: keeps the PE busy (HAM clock gate open)
            # from ~program start until the first weight triple lands ------
            ps_warm = psp.tile([P, 512], f32, tag="warm")
            for _ in range(36):
                nc.tensor.matmul(ps_warm[:, :], lhsT=wz_t[:, 0:P],
                                 rhs=wz_t[:, :], start=True, stop=True)

            # ---- conv1: 576 -> 256 over 7 rows, triple-streamed ----------
            ps1s = [psp.tile([P, N1], f32, tag="cva", name="ps1a"),
                    psp.tile([P, N1], f32, tag="cvb", name="ps1b")]

            def c1mm(w_t, col, tap, start, stop, half=False):
                ky, kx = tap // 3, tap % 3
                off = ky * W36 + kx + 1
                for mc in range(MC):
                    if half:
                        lhs = w_t[0:64, col + mc * P: col + mc * P + P]
                        r = xh_t[0:64, off: off + N1]
                    else:
                        kc = (col // TB1) % KC1
                        lhs = w_t[:, col + mc * P: col + mc * P + P]
                        r = x_t[:, kc * XBLK + off: kc * XBLK + off + N1]
                    nc.tensor.matmul(ps1s[mc][:, :], lhsT=lhs, rhs=r,
                                     start=start, stop=stop)

            first = True
            for tr in (2, 1, 0):           # triples in expected arrival order
                for tl in range(3):
                    for kc in range(KC1):
                        c1mm(w1_ts[tr], (tl * KC1 + kc) * TB1, tr * 3 + tl,
                             first, False)
                        first = False
            for tap in range(9):           # K=64 chunk, input ch 512..575
                c1mm(wh_t, tap * TB1, tap, False, tap == 8, half=True)

            for mc in range(MC):
                # ReLU(x + b) into the valid 32-wide spans of padded h1 rows
                ps1 = ps1s[mc]
                src = bass.AP(ps1.tensor, ps1.offset,
                              [[N1, P], [W36, R1], [1, 32]])
                dstb = h1_t[:, :]
                dst = bass.AP(dstb.tensor, dstb.offset + mc * H1BLK + 2,
                              [[MC * H1BLK + H1SLACK, P], [W36, R1], [1, 32]])
                nc.scalar.activation(dst, src, Relu, bias=b1_ap(mc))

            # zero h1 rows outside the global image (cores 0 and 7)
            for mc in range(MC):
                h1b = h1_t[:, :]
                cbb = cb_t[:, :]
                mask = bass.AP(cbb.tensor, cbb.offset + 58,
                               [[CB, P], [1, R1], [0, W36]])
                h1ap3 = bass.AP(h1b.tensor, h1b.offset + mc * H1BLK,
                                [[MC * H1BLK + H1SLACK, P], [W36, R1], [1, W36]])
                nc.vector.tensor_mul(h1ap3, h1ap3, mask)

            # ---- conv2: 256 -> 256 over 5 rows, triple-streamed ----------
            ps2s = [psp.tile([P, N2], f32, tag="cva", name="ps2a"),
                    psp.tile([P, N2], f32, tag="cvb", name="ps2b")]
            for tr in range(3):
                for tl in range(3):
                    tap = tr * 3 + tl
                    ky, kx = tap // 3, tap % 3
                    off = ky * W36 + kx + 1
                    for kc in range(KC2):
                        for mc in range(MC):
                            nc.tensor.matmul(
                                ps2s[mc][:, :],
                                lhsT=w2_ts[tr][:, (tl * KC2 + kc) * MC * P
                                               + mc * P:
                                               (tl * KC2 + kc) * MC * P
                                               + mc * P + P],
                                rhs=h1_t[:, kc * H1BLK + off:
                                         kc * H1BLK + off + N2],
                                start=(tap == 0 and kc == 0),
                                stop=(tap == 8 and kc == KC2 - 1),
                            )

            # h2 = relu(conv2 + b2) on the DVE
            for mc in range(MC):
                ps2 = ps2s[mc]
                src2 = bass.AP(ps2.tensor, ps2.offset,
                               [[N2, P], [W36, R2], [1, 32]])
                h2b = h2_t[:, :]
                dst2 = bass.AP(h2b.tensor, h2b.offset + mc * NV2,
                               [[MC * NV2, P], [W36, R2], [1, 32]])
                nc.vector.tensor_scalar(dst2, src2, b2_ap(mc), 0.0,
                                        Alu.add, Alu.max)

            # ---- 1x1 conv 256 -> 24 into (w, (h, c)) h-major -------------
            psr = psp.tile([32, R2 * OUT_C], f32, tag="psr")
            for h in range(R2):
                for kc in range(KC2):
                    nc.tensor.matmul(
                        psr[:, h * OUT_C:(h + 1) * OUT_C],
                        lhsT=h2_t[:, kc * NV2 + h * W36: kc * NV2 + h * W36 + 32],
                        rhs=cb_t[:, 10 + kc * OUT_C: 10 + (kc + 1) * OUT_C],
                        start=(kc == 0), stop=(kc == KC2 - 1),
                    )
            nc.vector.tensor_copy(rt_t[:, :], psr[:, :])
            # rts[w, 24h+c] = r[h] + r[h+1] for the averaged boundary rows
            nc.vector.tensor_tensor(rts_t[:, :], rt_t[:, 0:4 * OUT_C],
                                    rt_t[:, OUT_C:5 * OUT_C], Alu.add)

            # ---- W expansion (full width) --------------------------------
            psw = psp.tile([96, OUT_W], f32, tag="psw")
            psa = psp.tile([96, OUT_W], f32, tag="psa")
            for j in range(2):
                nc.tensor.matmul(psw[:, j * 512:(j + 1) * 512],
                                 lhsT=rt_t[:, 0:96],
                                 rhs=em_t[:, j * 512:(j + 1) * 512],
                                 start=True, stop=True)
            for j in range(2):
                nc.tensor.matmul(psa[:, j * 512:(j + 1) * 512],
                                 lhsT=rts_t[:, :],
                                 rhs=em_t[:, j * 512:(j + 1) * 512],
                                 start=True, stop=True)
            # rw = psw + br first (gates the HWDGE writes); replicate next
            # (gates the gpsimd write); av last (gates only the final DMA)
            nc.vector.tensor_scalar(rw_t[:, :], psw[:, :], br96, None, Alu.add)

            # ---- H replication writes (hybrid): stride-0-source writes cap
            # ~230 GB/s total (engine M2S stalls on repeated reads), so rows
            # 0-13 go stride-0 on the two HWDGE queues while rows 14-20 are
            # DVE-materialized into rw7 and written contiguously (16KB
            # descriptors, ~346 GB/s) on the gpsimd queue + averaged row --
            rwb = rw_t[:, :]
            avb = av_t[:, :]
            r7b = rw7_t[:, :]
            nc.vector.tensor_copy(
                bass.AP(r7b.tensor, r7b.offset,
                        [[7 * OUT_W, 96], [OUT_W, 7], [1, OUT_W]]),
                bass.AP(rwb.tensor, rwb.offset,
                        [[OUT_W, 96], [0, 7], [1, OUT_W]]))
            s7 = bass.AP(rwb.tensor, rwb.offset, [[OUT_W, 96], [0, 7], [1, OUT_W]])
            for qi, eng in enumerate((nc.sync, nc.scalar)):
                dst = bass.AP(out_d, qi * 7 * OUT_W,
                              [[RUN * OUT_W, 96], [OUT_W, 7], [1, OUT_W]])
                eng.dma_start(dst, s7)
            nc.vector.tensor_scalar(av_t[:, :], psa[:, :], 0.5, br96,
                                    Alu.mult, Alu.add)
            dst7 = bass.AP(out_d, 14 * OUT_W,
                           [[RUN * OUT_W, 96], [1, 7 * OUT_W]])
            nc.gpsimd.dma_start(dst7, bass.AP(r7b.tensor, r7b.offset,
                                              [[7 * OUT_W, 96], [1, 7 * OUT_W]]))
            srca = bass.AP(avb.tensor, avb.offset, [[OUT_W, 96], [1, OUT_W]])
            dsta = bass.AP(out_d, 21 * OUT_W, [[RUN * OUT_W, 96], [1, OUT_W]])
            nc.gpsimd.dma_start(dsta, srca)

    nc.compile()
    return nc


def _pack_inputs(x, w1, b1, w2, b2, wr, br):
    x = np.asarray(x, np.float32)
    w1 = np.asarray(w1, np.float32)
    w2 = np.asarray(w2, np.float32)
    wr = np.asarray(wr, np.float32)
    b1 = np.asarray(b1, np.float32)
    b2 = np.asarray(b2, np.float32)
    br = np.asarray(br, np.float32)

    xv = x[0]  # (576, 32, 32)
    xp = np.zeros((NCORES, P, KC1, RX, W36), np.float16)
    xhp = np.zeros((NCORES, 64, RX, W36), np.float16)
    for k in range(NCORES):
        for r in range(RX):
            g = 4 * k - 2 + r
            if 0 <= g < H:
                blkv = xv[:, g, :]  # (576, 32)
                xp[k, :, :, r, 2:34] = blkv[:512].reshape(KC1, P, W).transpose(1, 0, 2)
                xhp[k, :, r, 2:34] = blkv[512:]
    xp = xp.reshape(NCORES, P, KC1 * XBLK)
    xp = np.concatenate([xp, np.zeros((NCORES, P, XSLACK), np.float16)], axis=2)
    xhp = xhp.reshape(NCORES, 64, XBLK)
    xhp = np.concatenate([xhp, np.zeros((NCORES, 64, XSLACK), np.float16)],
                         axis=2)

    # w1 full chunks: [p, tap, kc, mc, m] = w1[mc*128+m, kc*128+p, ky, kx]
    w1v = w1.transpose(2, 3, 1, 0).reshape(9, IN_C, MID_C)  # (tap, ci, co)
    w1p = (w1v[:, :512, :].reshape(9, KC1, P, MC, P)
           .transpose(2, 0, 1, 3, 4).reshape(P, 9 * KC1 * TB1))
    w1ts = [np.ascontiguousarray(w1p[:, i * 3 * KC1 * TB1:(i + 1) * 3 * KC1 * TB1],
                                 np.float16) for i in range(3)]
    whp = (w1v[:, 512:, :].reshape(9, 64, MC, P)
           .transpose(1, 0, 2, 3).reshape(64, 9 * TB1))
    whp = np.ascontiguousarray(whp, np.float16)

    w2v = w2.transpose(2, 3, 1, 0).reshape(9, MID_C, MID_C)
    w2p = (w2v.reshape(9, KC2, P, MC, P).transpose(2, 0, 1, 3, 4)
           .reshape(P, 9 * TB2))
    w2ts = [np.ascontiguousarray(w2p[:, i * 3 * TB2:(i + 1) * 3 * TB2],
                                 np.float16) for i in range(3)]

    wrp = wr.T.reshape(KC2, P, OUT_C).transpose(1, 0, 2).reshape(P, KC2 * OUT_C)
    bbf = np.zeros((P, 5), np.float32)
    bbf[:, 0:2] = b1.reshape(MC, P).T
    bbf[:, 2:4] = b2.reshape(MC, P).T
    bbf[:96, 4] = np.tile(br, 4)          # br[p % 24] for p = 24h + c
    cbp = np.zeros((NCORES, P, CB), np.float16)
    cbp[:, :, 0:10] = bbf.view(np.float16)[None]
    cbp[:, :, 10:58] = wrp.astype(np.float16)[None]
    for k in range(NCORES):
        for r in range(R1):
            if 0 <= 4 * k - 1 + r < H:
                cbp[k, :, 58 + r] = 1.0

    em = (np.arange(OUT_W) // 32 == np.arange(32)[:, None]).astype(np.float16)

    shared = dict(whp=whp, emp=em)
    for i in range(3):
        shared[f"w1t{i}"] = w1ts[i]
        shared[f"w2t{i}"] = w2ts[i]
    in_maps = []
    for k in range(NCORES):
        m = dict(shared)
        m["xs"] = np.ascontiguousarray(xp[k])
        m["xh"] = np.ascontiguousarray(xhp[k])
        m["cbp"] = np.ascontiguousarray(cbp[k])
        in_maps.append(m)
    return in_maps


def kernel(x, w1, b1, w2, b2, wr, br):
    from concourse.bass_utils import run_bass_kernel_spmd

    if "nc" not in _prog_cache:
        _prog_cache["nc"] = _build_program()
    nc = _prog_cache["nc"]

    in_maps = _pack_inputs(x, w1, b1, w2, b2, wr, br)
    res = run_bass_kernel_spmd(nc, in_maps, list(range(NCORES)))

    _, t = _h_runs()
    out = np.empty((1, OUT_C, OUT_H, OUT_W), np.float32)
    for k in range(NCORES):
        # (4*24*22, 1024) rows ordered (h, c, run) h-major
        buf = res.results[k]["outb"].astype(np.float32)
        buf = buf.reshape(4, OUT_C, RUN, OUT_W)
        for hl in range(4):
            h = 4 * k + hl
            n = t[h + 1] - t[h]
            if h < H - 1:
                out[0, :, t[h]:t[h] + n - 1, :] = buf[hl, :, :n - 1, :]
                out[0, :, t[h] + n - 1, :] = buf[hl, :, RUN - 1, :]
            else:
                out[0, :, t[h]:t[h] + n, :] = buf[hl, :, :n, :]
    return out
